# revision 14
# baseline (speedup 1.0000x reference)
"""AttentionScore kernel for 8 TRN2 NeuronCores.

out[b, 0, g] = where(mask[b, g], -inf, 10*tanh((q[b] . k[b, g]) / 16))

Sharding: pure data parallel over the batch dim (2048 -> 8 x 256).

Per-core design (memory-bound, ~256 MiB of key data per core):
- batch lives on the SBUF partition axis; every partition computes dot
  products for its own batch row. No transpose, no TensorEngine, no PSUM.
- key is DMA-loaded with an inline f32->bf16 cast (SWDGE) at ~380 GB/s.
- DVE computes products (tensor_tensor bf16, 2x mode) and reduces ~60% of
  the g-columns via segmented tensor_reduce; the ScalarEngine reduces the
  other ~40% via activation(Identity, accum_out=...). The split keeps both
  engines just under the DMA time, so the kernel stays DMA-bound.
"""

import numpy as np

B, G, H = 2048, 1024, 256
NCORES = 8
BL = B // NCORES   # 256 batches per core
BB = 128           # batch block = partition count
GC = 32            # g-rows per K chunk (4 MiB f32 per DMA)
SUB = 8            # g-rows per DVE mul tile
C_CLIP = 10.0
SCALE = 1.0 / 16.0  # 1/sqrt(H)

# Of every SUB g-rows, this many are reduced on the ScalarEngine (ACT) and
# the rest with one segmented DVE tensor_reduce. avg 31/64 g to ACT.
ACT_PATTERN = (4, 4, 4, 4, 4, 4, 4, 3)

_CACHE = {}


def _build():
    import concourse.bass as bass
    import concourse.mybir as mybir
    from concourse import bacc, tile

    f32 = mybir.dt.float32
    bf16 = mybir.dt.bfloat16

    nc = bacc.Bacc(trn_type="TRN2", target_bir_lowering=False, num_devices=NCORES,
                   num_swdge_queues=4)
    q_ext = nc.declare_dram_parameter("query", [BL, 1, H], f32, isOutput=False)
    k_ext = nc.declare_dram_parameter("key", [BL, G, H], f32, isOutput=False)
    m_ext = nc.declare_dram_parameter("mask", [BL, G], mybir.dt.uint8, isOutput=False)
    o_ext = nc.declare_dram_parameter("out", [BL, 1, G], f32, isOutput=True)

    with tile.TileContext(nc) as tc:
        with (
            tc.tile_pool(name="kpool", bufs=6) as kpool,
            tc.tile_pool(name="prods", bufs=6) as prods_pool,
            tc.tile_pool(name="data", bufs=2) as data,
            tc.tile_pool(name="consts", bufs=1) as consts,
        ):
            neg_inf = consts.tile([BB, G], f32)
            nc.vector.memset(neg_inf[:], float("-inf"))

            for bb in range(BL // BB):
                bsl = slice(bb * BB, (bb + 1) * BB)
                q_tile = data.tile([BB, H], f32, tag="q")
                nc.sync.dma_start(q_tile[:], q_ext[bsl, 0, :])
                mask_u8 = data.tile([BB, G], mybir.dt.uint8, tag="m8")
                nc.sync.dma_start(mask_u8[:], m_ext[bsl, :])

                # q2 = bf16(q / 16), replicated SUB times along free dim
                q2 = data.tile([BB, H], bf16, tag="q2")
                nc.vector.tensor_scalar_mul(q2[:], q_tile[:], SCALE)
                q_rep = data.tile([BB, SUB * H], bf16, tag="qrep")
                for r in range(SUB):
                    nc.vector.tensor_copy(q_rep[:, r * H:(r + 1) * H], q2[:])

                u_tile = data.tile([BB, G], f32, tag="u")
                for gc in range(G // GC):
                    k_tile = kpool.tile([BB, GC * H], bf16, tag="ktile")
                    nc.gpsimd.dma_start(
                        k_tile[:], k_ext[bsl, gc * GC:(gc + 1) * GC, :], single_packet=True
                    )
                    for s in range(GC // SUB):
                        g0 = gc * GC + s * SUB
                        p_tile = prods_pool.tile([BB, SUB * H], bf16, tag="p")
                        nc.vector.tensor_tensor(
                            out=p_tile[:],
                            in0=k_tile[:, s * SUB * H:(s + 1) * SUB * H],
                            in1=q_rep[:],
                            op=mybir.AluOpType.mult,
                        )
                        n_act = ACT_PATTERN[s % len(ACT_PATTERN)]
                        for a in range(n_act):
                            nc.scalar.activation(
                                out=p_tile[:, a * H:(a + 1) * H],
                                in_=p_tile[:, a * H:(a + 1) * H],
                                func=mybir.ActivationFunctionType.Identity,
                                accum_out=u_tile[:, g0 + a:g0 + a + 1],
                            )
                        n_dve = SUB - n_act
                        if n_dve > 0:
                            seg = p_tile[:, n_act * H:].rearrange(
                                "p (j h) -> p j h", h=H
                            )
                            nc.vector.tensor_reduce(
                                out=u_tile[:, g0 + n_act:g0 + SUB],
                                in_=seg,
                                axis=mybir.AxisListType.X,
                                op=mybir.AluOpType.add,
                            )

                # epilogue: logits = 10*tanh(u); masked -> -inf
                t_tile = data.tile([BB, G], f32, tag="t")
                nc.scalar.activation(
                    t_tile[:], u_tile[:], mybir.ActivationFunctionType.Tanh
                )
                nc.vector.tensor_scalar_mul(t_tile[:], t_tile[:], C_CLIP)
                nc.vector.copy_predicated(t_tile[:], mask_u8[:], neg_inf[:])
                nc.sync.dma_start(o_ext[bsl, 0, :], t_tile[:])

    nc.compile()
    return nc


def _get_nc():
    if "nc" not in _CACHE:
        _CACHE["nc"] = _build()
    return _CACHE["nc"]


def kernel(query, key, mask):
    from concourse.bass_utils import run_bass_kernel_spmd
    import os

    query = np.ascontiguousarray(np.asarray(query, dtype=np.float32))
    key = np.ascontiguousarray(np.asarray(key, dtype=np.float32))
    mask_u8 = np.ascontiguousarray(np.asarray(mask).astype(np.uint8))

    nc = _get_nc()
    in_maps = []
    for i in range(NCORES):
        sl = slice(i * BL, (i + 1) * BL)
        in_maps.append({
            "query": query[sl],
            "key": key[sl],
            "mask": mask_u8[sl],
        })

    trace = bool(int(os.environ.get("KERNEL_TRACE", "0")))
    res = run_bass_kernel_spmd(nc, in_maps, list(range(NCORES)), trace=trace)
    _CACHE["last_result"] = res
    out = np.concatenate([res.results[i]["out"] for i in range(NCORES)], axis=0)
    return out


# revision 15
# speedup vs baseline: 1.0366x; 1.0366x over previous
"""AttentionScore kernel for 8 TRN2 NeuronCores.

out[b, 0, g] = where(mask[b, g], -inf, 10*tanh((q[b] . k[b, g]) / 16))

Sharding: pure data parallel over the batch dim (2048 -> 8 x 256).

Per-core design (memory-bound, ~256 MiB of key data per core):
- batch lives on the SBUF partition axis; every partition computes dot
  products for its own batch row. No transpose, no TensorEngine, no PSUM.
- key is DMA-loaded with an inline f32->bf16 cast (SWDGE) at ~380 GB/s.
- DVE computes products (tensor_tensor bf16, 2x mode) and reduces ~60% of
  the g-columns via segmented tensor_reduce; the ScalarEngine reduces the
  other ~40% via activation(Identity, accum_out=...). The split keeps both
  engines just under the DMA time, so the kernel stays DMA-bound.
"""

import numpy as np

B, G, H = 2048, 1024, 256
NCORES = 8
BL = B // NCORES   # 256 batches per core
BB = 128           # batch block = partition count
GC = 16            # g-rows per K chunk (2 MiB f32 per DMA)
SUB = 8            # g-rows per DVE mul tile
C_CLIP = 10.0
SCALE = 1.0 / 16.0  # 1/sqrt(H)

# Of every SUB g-rows, this many are reduced on the ScalarEngine (ACT) and
# the rest with one segmented DVE tensor_reduce. avg 31/64 g to ACT.
ACT_PATTERN = (4, 4, 4, 4, 4, 4, 4, 3)

_CACHE = {}


def _build():
    import concourse.bass as bass
    import concourse.mybir as mybir
    from concourse import bacc, tile

    f32 = mybir.dt.float32
    bf16 = mybir.dt.bfloat16

    nc = bacc.Bacc(trn_type="TRN2", target_bir_lowering=False, num_devices=NCORES,
                   num_swdge_queues=4)
    q_ext = nc.declare_dram_parameter("query", [BL, 1, H], f32, isOutput=False)
    k_ext = nc.declare_dram_parameter("key", [BL, G, H], f32, isOutput=False)
    m_ext = nc.declare_dram_parameter("mask", [BL, G], mybir.dt.uint8, isOutput=False)
    o_ext = nc.declare_dram_parameter("out", [BL, 1, G], f32, isOutput=True)

    with tile.TileContext(nc) as tc:
        with (
            tc.tile_pool(name="kpool", bufs=10) as kpool,
            tc.tile_pool(name="prods", bufs=6) as prods_pool,
            tc.tile_pool(name="data", bufs=2) as data,
            tc.tile_pool(name="consts", bufs=1) as consts,
        ):
            neg_inf = consts.tile([BB, G], f32)
            nc.vector.memset(neg_inf[:], float("-inf"))

            for bb in range(BL // BB):
                bsl = slice(bb * BB, (bb + 1) * BB)
                q_tile = data.tile([BB, H], f32, tag="q")
                nc.sync.dma_start(q_tile[:], q_ext[bsl, 0, :])
                mask_u8 = data.tile([BB, G], mybir.dt.uint8, tag="m8")
                nc.sync.dma_start(mask_u8[:], m_ext[bsl, :])

                # q2 = bf16(q / 16), replicated SUB times along free dim
                q2 = data.tile([BB, H], bf16, tag="q2")
                nc.vector.tensor_scalar_mul(q2[:], q_tile[:], SCALE)
                q_rep = data.tile([BB, SUB * H], bf16, tag="qrep")
                for r in range(SUB):
                    nc.vector.tensor_copy(q_rep[:, r * H:(r + 1) * H], q2[:])

                u_tile = data.tile([BB, G], f32, tag="u")
                for gc in range(G // GC):
                    k_tile = kpool.tile([BB, GC * H], bf16, tag="ktile")
                    nc.gpsimd.dma_start(
                        k_tile[:], k_ext[bsl, gc * GC:(gc + 1) * GC, :]
                    )
                    for s in range(GC // SUB):
                        g0 = gc * GC + s * SUB
                        p_tile = prods_pool.tile([BB, SUB * H], bf16, tag="p")
                        nc.vector.tensor_tensor(
                            out=p_tile[:],
                            in0=k_tile[:, s * SUB * H:(s + 1) * SUB * H],
                            in1=q_rep[:],
                            op=mybir.AluOpType.mult,
                        )
                        n_act = ACT_PATTERN[s % len(ACT_PATTERN)]
                        for a in range(n_act):
                            nc.scalar.activation(
                                out=p_tile[:, a * H:(a + 1) * H],
                                in_=p_tile[:, a * H:(a + 1) * H],
                                func=mybir.ActivationFunctionType.Identity,
                                accum_out=u_tile[:, g0 + a:g0 + a + 1],
                            )
                        n_dve = SUB - n_act
                        if n_dve > 0:
                            seg = p_tile[:, n_act * H:].rearrange(
                                "p (j h) -> p j h", h=H
                            )
                            nc.vector.tensor_reduce(
                                out=u_tile[:, g0 + n_act:g0 + SUB],
                                in_=seg,
                                axis=mybir.AxisListType.X,
                                op=mybir.AluOpType.add,
                            )

                # epilogue: logits = 10*tanh(u); masked -> -inf
                t_tile = data.tile([BB, G], f32, tag="t")
                nc.scalar.activation(
                    t_tile[:], u_tile[:], mybir.ActivationFunctionType.Tanh
                )
                nc.vector.tensor_scalar_mul(t_tile[:], t_tile[:], C_CLIP)
                nc.vector.copy_predicated(t_tile[:], mask_u8[:], neg_inf[:])
                nc.sync.dma_start(o_ext[bsl, 0, :], t_tile[:])

    nc.compile()
    return nc


def _get_nc():
    if "nc" not in _CACHE:
        _CACHE["nc"] = _build()
    return _CACHE["nc"]


def kernel(query, key, mask):
    from concourse.bass_utils import run_bass_kernel_spmd
    import os

    query = np.ascontiguousarray(np.asarray(query, dtype=np.float32))
    key = np.ascontiguousarray(np.asarray(key, dtype=np.float32))
    mask_u8 = np.ascontiguousarray(np.asarray(mask).astype(np.uint8))

    nc = _get_nc()
    in_maps = []
    for i in range(NCORES):
        sl = slice(i * BL, (i + 1) * BL)
        in_maps.append({
            "query": query[sl],
            "key": key[sl],
            "mask": mask_u8[sl],
        })

    trace = bool(int(os.environ.get("KERNEL_TRACE", "0")))
    res = run_bass_kernel_spmd(nc, in_maps, list(range(NCORES)), trace=trace)
    _CACHE["last_result"] = res
    out = np.concatenate([res.results[i]["out"] for i in range(NCORES)], axis=0)
    return out


# revision 16
# speedup vs baseline: 1.1985x; 1.1562x over previous
"""AttentionScore kernel for 8 TRN2 NeuronCores.

out[b, 0, g] = where(mask[b, g], -inf, 10*tanh((q[b] . k[b, g]) / 16))

Sharding: pure data parallel over the batch dim (2048 -> 8 x 256).

Per-core design (memory-bound, ~256 MiB of key data per core):
- batch lives on the SBUF partition axis; every partition computes dot
  products for its own batch row. No transpose, no TensorEngine, no PSUM.
- key is DMA-loaded with an inline f32->bf16 cast (SWDGE) at ~380 GB/s.
- DVE computes products (tensor_tensor bf16, 2x mode) and reduces ~60% of
  the g-columns via segmented tensor_reduce; the ScalarEngine reduces the
  other ~40% via activation(Identity, accum_out=...). The split keeps both
  engines just under the DMA time, so the kernel stays DMA-bound.
"""

import numpy as np

B, G, H = 2048, 1024, 256
NCORES = 8
BL = B // NCORES   # 256 batches per core
BB = 128           # batch block = partition count
GC = 32            # g-rows per K chunk (4 MiB f32 per DMA)
SUB = 8            # g-rows per DVE mul tile
C_CLIP = 10.0
SCALE = 1.0 / 16.0  # 1/sqrt(H)

# Of every SUB g-rows, this many are reduced on the ScalarEngine (ACT) and
# the rest with one segmented DVE tensor_reduce. avg 31/64 g to ACT.
ACT_PATTERN = (4, 4, 4, 4, 4, 4, 4, 3)

_CACHE = {}


def _build():
    import concourse.bass as bass
    import concourse.mybir as mybir
    from concourse import bacc, tile

    f32 = mybir.dt.float32
    bf16 = mybir.dt.bfloat16

    nc = bacc.Bacc(trn_type="TRN2", target_bir_lowering=False, num_devices=NCORES,
                   num_swdge_queues=4)
    q_ext = nc.declare_dram_parameter("query", [BL, 1, H], f32, isOutput=False)
    k_ext = nc.declare_dram_parameter("key", [BL, G, H], f32, isOutput=False)
    m_ext = nc.declare_dram_parameter("mask", [BL, G], mybir.dt.uint8, isOutput=False)
    o_ext = nc.declare_dram_parameter("out", [BL, 1, G], f32, isOutput=True)

    with tile.TileContext(nc) as tc:
        with (
            tc.tile_pool(name="kpool", bufs=4) as kpool,
            tc.tile_pool(name="prods", bufs=6) as prods_pool,
            tc.tile_pool(name="data", bufs=2) as data,
            tc.tile_pool(name="consts", bufs=1) as consts,
        ):
            neg_inf = consts.tile([BB, G], f32)
            nc.vector.memset(neg_inf[:], float("-inf"))

            for bb in range(BL // BB):
                bsl = slice(bb * BB, (bb + 1) * BB)
                q_tile = data.tile([BB, H], f32, tag="q")
                nc.sync.dma_start(q_tile[:], q_ext[bsl, 0, :])
                mask_u8 = data.tile([BB, G], mybir.dt.uint8, tag="m8")
                nc.sync.dma_start(mask_u8[:], m_ext[bsl, :])

                # q2 = bf16(q / 16), replicated SUB times along free dim
                q2 = data.tile([BB, H], bf16, tag="q2")
                nc.vector.tensor_scalar_mul(q2[:], q_tile[:], SCALE)
                q_rep = data.tile([BB, SUB * H], bf16, tag="qrep")
                for r in range(SUB):
                    nc.vector.tensor_copy(q_rep[:, r * H:(r + 1) * H], q2[:])

                u_tile = data.tile([BB, G], f32, tag="u")
                for gc in range(G // GC):
                    k_tile = kpool.tile([BB, GC * H], bf16, tag="ktile")
                    nc.gpsimd.dma_start(
                        k_tile[:], k_ext[bsl, gc * GC:(gc + 1) * GC, :]
                    )
                    for s in range(GC // SUB):
                        g0 = gc * GC + s * SUB
                        p_tile = prods_pool.tile([BB, SUB * H], bf16, tag="p")
                        nc.vector.tensor_tensor(
                            out=p_tile[:],
                            in0=k_tile[:, s * SUB * H:(s + 1) * SUB * H],
                            in1=q_rep[:],
                            op=mybir.AluOpType.mult,
                        )
                        n_act = ACT_PATTERN[s % len(ACT_PATTERN)]
                        for a in range(n_act):
                            nc.scalar.activation(
                                out=p_tile[:, a * H:(a + 1) * H],
                                in_=p_tile[:, a * H:(a + 1) * H],
                                func=mybir.ActivationFunctionType.Identity,
                                accum_out=u_tile[:, g0 + a:g0 + a + 1],
                            )
                        n_dve = SUB - n_act
                        if n_dve > 0:
                            seg = p_tile[:, n_act * H:].rearrange(
                                "p (j h) -> p j h", h=H
                            )
                            nc.vector.tensor_reduce(
                                out=u_tile[:, g0 + n_act:g0 + SUB],
                                in_=seg,
                                axis=mybir.AxisListType.X,
                                op=mybir.AluOpType.add,
                            )

                # epilogue: logits = 10*tanh(u); masked -> -inf
                t_tile = data.tile([BB, G], f32, tag="t")
                nc.scalar.activation(
                    t_tile[:], u_tile[:], mybir.ActivationFunctionType.Tanh
                )
                nc.vector.tensor_scalar_mul(t_tile[:], t_tile[:], C_CLIP)
                nc.vector.copy_predicated(t_tile[:], mask_u8[:], neg_inf[:])
                nc.sync.dma_start(o_ext[bsl, 0, :], t_tile[:])

    nc.compile()
    return nc


def _get_nc():
    if "nc" not in _CACHE:
        _CACHE["nc"] = _build()
    return _CACHE["nc"]


def kernel(query, key, mask):
    from concourse.bass_utils import run_bass_kernel_spmd
    import os

    query = np.ascontiguousarray(np.asarray(query, dtype=np.float32))
    key = np.ascontiguousarray(np.asarray(key, dtype=np.float32))
    mask_u8 = np.ascontiguousarray(np.asarray(mask).astype(np.uint8))

    nc = _get_nc()
    in_maps = []
    for i in range(NCORES):
        sl = slice(i * BL, (i + 1) * BL)
        in_maps.append({
            "query": query[sl],
            "key": key[sl],
            "mask": mask_u8[sl],
        })

    trace = bool(int(os.environ.get("KERNEL_TRACE", "0")))
    res = run_bass_kernel_spmd(nc, in_maps, list(range(NCORES)), trace=trace)
    _CACHE["last_result"] = res
    out = np.concatenate([res.results[i]["out"] for i in range(NCORES)], axis=0)
    return out
